# revision 4
# baseline (speedup 1.0000x reference)
"""Trainium2 Bass kernel for nn_ATLModule (few-shot cosine-attention scoring).

Strategy: data-parallel over the 64 query images (8 per NeuronCore).
Support tensor + tiny MLP weights replicated on every core.

Per core (q=8 local queries, p=800 query pixels, m=2500 support pixels, c=640):
  - Load query/support in natural [c, pixels] layout (c on partitions).
  - Support/query L2 norms: ACT Square (bf16) + ones-matmul column reduce,
    DRAM round-trip reshape to per-partition layout, sqrt + exact reciprocal.
  - Raw Gram matrix G = qT.T @ sn_normalized via fp32r matmuls (full-rate PE),
    query norm folded into the per-partition ACT scale.
  - cf = Sigmoid(scale_p * G + bias_p) on ScalarE with fused accum -> l1 row sums.
  - Per-support segment sums of cf*G via fused DVE tensor_tensor_reduce.
  - final_local = (1/|q_p|) * seg / max(l1, 1e-12); final_score via 0/1
    selection matmul + *0.3 (mean over 100 pixels * scale 30).
"""
import numpy as np

Q, S, C, H, W = 64, 25, 640, 10, 10
HW = H * W                    # 100
NCORES = 8
QL = Q // NCORES              # 8 queries per core
PL = QL * HW                  # 800 query-pixel rows per core
M = S * HW                    # 2500 support columns
KC = C // 128                 # 5 contraction chunks
NCH = 500                     # support columns per psum chunk (5 supports)
NJ = M // NCH                 # 5 chunks
NT = (PL + 127) // 128        # 7 query-pixel row tiles
SCALE = 30.0
ATT_SCALE = 50.0
FROM_VALUE = 0.5
VALUE_INTERVAL = 0.3
EPS = 1e-12

_CACHE = {}


def _build_nc():
    import concourse.bass as bass
    import concourse.tile as tile
    from concourse import bacc, mybir
    from contextlib import ExitStack

    F32 = mybir.dt.float32
    F32R = mybir.dt.float32r
    BF16 = mybir.dt.bfloat16
    AF = mybir.ActivationFunctionType
    ALU = mybir.AluOpType

    nc = bacc.Bacc("TRN2", target_bir_lowering=False, debug=False,
                   num_devices=NCORES)

    q_d = nc.dram_tensor("q", [QL, C, HW], F32R, kind="ExternalInput")
    s_d = nc.dram_tensor("s", [S, C, HW], F32R, kind="ExternalInput")
    w1t_d = nc.dram_tensor("w1t", [C, 40], F32R, kind="ExternalInput")
    b1_d = nc.dram_tensor("b1", [40], F32, kind="ExternalInput")
    w2_d = nc.dram_tensor("w2", [40], F32, kind="ExternalInput")
    b2_d = nc.dram_tensor("b2", [1], F32, kind="ExternalInput")
    qsel_d = nc.dram_tensor("qsel", [NT * 128, QL], F32, kind="ExternalInput")
    fl_d = nc.dram_tensor("flocal", [PL, S], F32, kind="ExternalOutput")
    fs_d = nc.dram_tensor("fscore", [QL, S], F32, kind="ExternalOutput")

    def bcast_ap(handle, n):
        ap = handle.ap()
        return bass.AP(tensor=ap.tensor, offset=ap.offset, ap=[[0, 128], [1, n]])

    with tile.TileContext(nc) as tc, ExitStack() as ctx:
        big = ctx.enter_context(tc.tile_pool(name="big", bufs=1))
        small = ctx.enter_context(tc.tile_pool(name="small", bufs=1))
        work = ctx.enter_context(tc.tile_pool(name="work", bufs=3))
        flp = ctx.enter_context(tc.tile_pool(name="flp", bufs=NT))
        segp = ctx.enter_context(tc.tile_pool(name="segp", bufs=2))
        psum_m = ctx.enter_context(tc.tile_pool(name="psm", bufs=3, space="PSUM"))
        psum_s = ctx.enter_context(tc.tile_pool(name="pss", bufs=1, space="PSUM"))

        # ---------------- loads ----------------
        qT = big.tile([128, KC, PL], F32R)        # [c128, kc, (q hw)]
        for k in range(KC):
            nc.sync.dma_start(
                qT[:, k, :],
                bass.AP(tensor=q_d.ap().tensor, offset=k * 128 * HW,
                        ap=[[HW, 128], [C * HW, QL], [1, HW]]))
        sn = big.tile([128, KC, M], F32R)         # raw support [c128, kc, (s hw)]
        for k in range(KC):
            nc.sync.dma_start(
                sn[:, k, :],
                bass.AP(tensor=s_d.ap().tensor, offset=k * 128 * HW,
                        ap=[[HW, 128], [C * HW, S], [1, HW]]))
        w1t = small.tile([128, KC, 40], F32R)
        nc.sync.dma_start(w1t, w1t_d.rearrange("(kc p) j -> p kc j", p=128))
        qsel = small.tile([128, NT, QL], F32)
        nc.sync.dma_start(qsel, qsel_d.rearrange("(t p) j -> p t j", p=128))
        b1b = small.tile([128, 40], F32)
        nc.sync.dma_start(b1b, bcast_ap(b1_d, 40))
        w2b = small.tile([128, 40], F32)
        nc.sync.dma_start(w2b, bcast_ap(w2_d, 40))
        b2b = small.tile([128, 1], F32)
        nc.sync.dma_start(b2b, bcast_ap(b2_d, 1))
        ones_bf = small.tile([128, 1], BF16)
        nc.vector.memset(ones_bf, 1.0)
        zero11 = small.tile([128, 1], F32)
        nc.vector.memset(zero11, 0.0)

        # ---------------- support norms ----------------
        # all-ones [128,128] lhsT: column-sum AND broadcast to all partitions
        sq = big.tile([128, KC, M], BF16)
        for k in range(KC):
            nc.scalar.activation(out=sq[:, k, :], in_=sn[:, k, :].bitcast(F32),
                                 func=AF.Square)
        allones = small.tile([128, 128], BF16)
        nc.vector.memset(allones, 1.0)
        rn_bc = big.tile([128, M], F32)
        lnt = work.tile([128, NCH], F32, tag="lnt")
        for j in range(NJ):
            bc_ps = psum_s.tile([128, 512], F32, tag="bc")
            for k in range(KC):
                nc.tensor.matmul(bc_ps[:, :NCH], lhsT=allones,
                                 rhs=sq[:, k, j * NCH:(j + 1) * NCH],
                                 start=(k == 0), stop=(k == KC - 1))
            lnt = work.tile([128, NCH], F32, tag="lnt")
            nc.scalar.activation(out=lnt, in_=bc_ps[:, :NCH], func=AF.Ln,
                                 bias=zero11)
            nc.scalar.activation(out=rn_bc[:, j * NCH:(j + 1) * NCH], in_=lnt,
                                 func=AF.Exp, bias=zero11, scale=-0.5)
        sn_n = big.tile([128, KC, M], F32R)
        for k in range(KC):
            nc.vector.tensor_mul(sn_n[:, k, :], sn[:, k, :].bitcast(F32), rn_bc)

        # ---------------- query norms ----------------
        sqq = big.tile([128, KC, PL], BF16)
        for k in range(KC):
            nc.scalar.activation(out=sqq[:, k, :], in_=qT[:, k, :].bitcast(F32),
                                 func=AF.Square)
        arow = small.tile([128, 896], F32)
        nc.vector.memset(arow, 1.0)
        for j2 in range(2):
            bc_ps = psum_s.tile([128, 512], F32, tag="bc")
            for k in range(KC):
                nc.tensor.matmul(bc_ps[:, :400], lhsT=allones,
                                 rhs=sqq[:, k, j2 * 400:(j2 + 1) * 400],
                                 start=(k == 0), stop=(k == KC - 1))
            nc.scalar.copy(arow[:, j2 * 400:(j2 + 1) * 400], bc_ps[:, :400])
        from concourse.masks import make_identity
        ident = small.tile([128, 128], F32)
        make_identity(nc, ident)
        a2pre = small.tile([128, NT], F32)
        for t in range(NT):
            tr_ps = psum_s.tile([128, 128], F32, tag="tr")
            nc.tensor.transpose(tr_ps, arow[:, t * 128:(t + 1) * 128], ident)
            nc.scalar.copy(a2pre[:, t:t + 1], tr_ps[:, 0:1])
        lnq = small.tile([128, NT], F32)
        nc.scalar.activation(out=lnq, in_=a2pre, func=AF.Ln, bias=zero11)
        a_r = small.tile([128, NT], F32)
        nc.scalar.activation(out=a_r, in_=lnq, func=AF.Exp, bias=zero11,
                             scale=-0.5)

        # ---------------- psi / adaptive threshold ----------------
        pp = small.tile([128, NT], F32)
        for t in range(NT):
            rt = min(128, PL - t * 128)
            hid_ps = psum_s.tile([128, 40], F32, tag="hid")
            for k in range(KC):
                nc.tensor.matmul(hid_ps[:rt, :40],
                                 lhsT=qT[:, k, t * 128:t * 128 + rt],
                                 rhs=w1t[:, k, :],
                                 start=(k == 0), stop=(k == KC - 1))
            hid = work.tile([128, 40], F32, tag="hid_sb")
            nc.vector.scalar_tensor_tensor(
                out=hid[:rt], in0=hid_ps[:rt, :40], scalar=a_r[:rt, t:t + 1],
                in1=b1b[:rt], op0=ALU.mult, op1=ALU.add)
            hl = work.tile([128, 40], F32, tag="hl_sb")
            nc.scalar.activation(out=hl[:rt], in_=hid[:rt], func=AF.Lrelu,
                                 bias=zero11[:rt], scale=1.0, alpha=0.2)
            scr40 = work.tile([128, 40], F32, tag="scr40")
            nc.vector.affine_mul_reduce(
                out=scr40[:rt], accum_out=pp[:rt, t:t + 1],
                in0=hl[:rt], in1=w2b[:rt], scale=1.0, bias=0.0)
        sigp = small.tile([128, NT], F32)
        nc.scalar.activation(out=sigp, in_=pp, func=AF.Sigmoid, bias=b2b,
                             scale=1.0)
        bias_sb = small.tile([128, NT], F32)
        nc.vector.tensor_scalar(out=bias_sb, in0=sigp,
                                scalar1=-ATT_SCALE * VALUE_INTERVAL,
                                scalar2=-ATT_SCALE * FROM_VALUE,
                                op0=ALU.mult, op1=ALU.add)
        scale_sb = small.tile([128, NT], F32)
        nc.vector.tensor_scalar(out=scale_sb, in0=a_r, scalar1=ATT_SCALE,
                                scalar2=None, op0=ALU.mult)

        # ---------------- main loop ----------------
        fl_tiles = []
        for t in range(NT):
            rt = min(128, PL - t * 128)
            l1p = work.tile([128, NJ], F32, tag="l1p")
            nc.vector.memset(l1p, 0.0)
            seg = segp.tile([128, S], F32, tag="seg")
            nc.vector.memset(seg, 0.0)
            for j in range(NJ):
                g_ps = psum_m.tile([128, 512], F32, tag="gps")
                for k in range(KC):
                    nc.tensor.matmul(g_ps[:rt, :NCH],
                                     lhsT=qT[:, k, t * 128:t * 128 + rt],
                                     rhs=sn_n[:, k, j * NCH:(j + 1) * NCH],
                                     start=(k == 0), stop=(k == KC - 1))
                cf = work.tile([128, NCH], F32, tag="cf")
                nc.scalar.activation(out=cf[:rt], in_=g_ps[:rt, :NCH],
                                     func=AF.Sigmoid,
                                     bias=bias_sb[:rt, t:t + 1],
                                     scale=scale_sb[:rt, t:t + 1],
                                     accum_out=l1p[:rt, j:j + 1])
                for s5 in range(NCH // HW):
                    col = j * (NCH // HW) + s5
                    vscr = work.tile([128, HW], F32, tag="vscr")
                    nc.vector.affine_mul_reduce(
                        out=vscr[:rt], accum_out=seg[:rt, col:col + 1],
                        in0=cf[:rt, s5 * HW:(s5 + 1) * HW],
                        in1=g_ps[:rt, s5 * HW:(s5 + 1) * HW],
                        scale=1.0, bias=0.0)
            l1v = work.tile([128, 1], F32, tag="l1v")
            nc.vector.tensor_reduce(out=l1v, in_=l1p,
                                    axis=mybir.AxisListType.X, op=ALU.add)
            nc.vector.tensor_scalar_max(out=l1v, in0=l1v, scalar1=EPS)
            l1r = work.tile([128, 1], F32, tag="l1r")
            nc.vector.reciprocal(l1r, l1v)
            afac = work.tile([128, 1], F32, tag="afac")
            nc.vector.tensor_scalar(out=afac, in0=l1r,
                                    scalar1=a_r[:, t:t + 1], scalar2=None,
                                    op0=ALU.mult)
            flt = flp.tile([128, S], F32, tag="flt")
            nc.vector.tensor_scalar(out=flt, in0=seg, scalar1=afac,
                                    scalar2=None, op0=ALU.mult)
            nc.sync.dma_start(fl_d[t * 128:t * 128 + rt, :], flt[:rt])
            fl_tiles.append(flt)

        # ---------------- final score ----------------
        fs_ps = psum_s.tile([QL, 512], F32, tag="fsps")
        for t in range(NT):
            nc.tensor.matmul(fs_ps[:QL, :S], lhsT=qsel[:, t, :],
                             rhs=fl_tiles[t], start=(t == 0),
                             stop=(t == NT - 1))
        fs_sb = small.tile([QL, S], F32)
        nc.vector.tensor_scalar(out=fs_sb, in0=fs_ps[:QL, :S],
                                scalar1=SCALE / HW, scalar2=None, op0=ALU.mult)
        nc.sync.dma_start(fs_d.ap(), fs_sb)

    nc.compile()
    return nc


def _get_nc():
    if "nc" not in _CACHE:
        _CACHE["nc"] = _build_nc()
    return _CACHE["nc"]


def _qsel_host():
    qsel = np.zeros((NT * 128, QL), dtype=np.float32)
    rows = np.arange(PL)
    qsel[rows, rows // HW] = 1.0
    return qsel


def kernel(query_data, support_data, w1, b1, w2, b2):
    from concourse.bass_utils import run_bass_kernel_spmd

    query_data = np.asarray(query_data, dtype=np.float32)
    support_data = np.asarray(support_data, dtype=np.float32)
    q_resh = np.ascontiguousarray(query_data.reshape(Q, C, HW))
    s_resh = np.ascontiguousarray(support_data.reshape(S, C, HW))
    w1t = np.ascontiguousarray(np.asarray(w1, dtype=np.float32).T)     # [640, 40]
    b1 = np.ascontiguousarray(np.asarray(b1, dtype=np.float32).reshape(40))
    w2v = np.ascontiguousarray(np.asarray(w2, dtype=np.float32).reshape(40))
    b2 = np.ascontiguousarray(np.asarray(b2, dtype=np.float32).reshape(1))
    qsel = _qsel_host()

    nc = _get_nc()
    in_maps = []
    for i in range(NCORES):
        in_maps.append({
            "q": np.ascontiguousarray(q_resh[i * QL:(i + 1) * QL]),
            "s": s_resh, "w1t": w1t, "b1": b1, "w2": w2v, "b2": b2,
            "qsel": qsel,
        })
    res = run_bass_kernel_spmd(nc, in_maps, list(range(NCORES)))
    _CACHE["last_results"] = res

    fl = np.concatenate([r["flocal"] for r in res.results], axis=0)  # [6400, 25]
    fs = np.concatenate([r["fscore"] for r in res.results], axis=0)  # [64, 25]
    final_local = np.ascontiguousarray(
        fl.reshape(Q, HW, S).transpose(0, 2, 1)).astype(np.float32)
    return fs.astype(np.float32), final_local


# revision 5
# speedup vs baseline: 1.0014x; 1.0014x over previous
"""Trainium2 Bass kernel for nn_ATLModule (few-shot cosine-attention scoring).

Strategy: data-parallel over the 64 query images (8 per NeuronCore).
Support tensor + tiny MLP weights replicated on every core.

Per core (q=8 local queries, p=800 query pixels, m=2500 support pixels, c=640):
  - Load query/support in natural [c, pixels] layout (c on partitions).
  - Support/query L2 norms: ACT Square (bf16) + ones-matmul column reduce,
    DRAM round-trip reshape to per-partition layout, sqrt + exact reciprocal.
  - Raw Gram matrix G = qT.T @ sn_normalized via fp32r matmuls (full-rate PE),
    query norm folded into the per-partition ACT scale.
  - cf = Sigmoid(scale_p * G + bias_p) on ScalarE with fused accum -> l1 row sums.
  - Per-support segment sums of cf*G via fused DVE tensor_tensor_reduce.
  - final_local = (1/|q_p|) * seg / max(l1, 1e-12); final_score via 0/1
    selection matmul + *0.3 (mean over 100 pixels * scale 30).
"""
import numpy as np

Q, S, C, H, W = 64, 25, 640, 10, 10
HW = H * W                    # 100
NCORES = 8
QL = Q // NCORES              # 8 queries per core
PL = QL * HW                  # 800 query-pixel rows per core
M = S * HW                    # 2500 support columns
KC = C // 128                 # 5 contraction chunks
NCH = 500                     # support columns per psum chunk (5 supports)
NJ = M // NCH                 # 5 chunks
NT = (PL + 127) // 128        # 7 query-pixel row tiles
SCALE = 30.0
ATT_SCALE = 50.0
FROM_VALUE = 0.5
VALUE_INTERVAL = 0.3
EPS = 1e-12

_CACHE = {}


def _build_nc():
    import concourse.bass as bass
    import concourse.tile as tile
    from concourse import bacc, mybir
    from contextlib import ExitStack

    F32 = mybir.dt.float32
    F32R = mybir.dt.float32r
    BF16 = mybir.dt.bfloat16
    AF = mybir.ActivationFunctionType
    ALU = mybir.AluOpType

    nc = bacc.Bacc("TRN2", target_bir_lowering=False, debug=False,
                   num_devices=NCORES)

    q_d = nc.dram_tensor("q", [QL, C, HW], F32R, kind="ExternalInput")
    s_d = nc.dram_tensor("s", [S, C, HW], F32R, kind="ExternalInput")
    w1t_d = nc.dram_tensor("w1t", [C, 40], F32R, kind="ExternalInput")
    b1_d = nc.dram_tensor("b1", [40], F32, kind="ExternalInput")
    w2_d = nc.dram_tensor("w2", [40], F32, kind="ExternalInput")
    b2_d = nc.dram_tensor("b2", [1], F32, kind="ExternalInput")
    qsel_d = nc.dram_tensor("qsel", [NT * 128, QL], F32, kind="ExternalInput")
    fl_d = nc.dram_tensor("flocal", [PL, S], F32, kind="ExternalOutput")
    fs_d = nc.dram_tensor("fscore", [QL, S], F32, kind="ExternalOutput")

    def bcast_ap(handle, n):
        ap = handle.ap()
        return bass.AP(tensor=ap.tensor, offset=ap.offset, ap=[[0, 128], [1, n]])

    with tile.TileContext(nc) as tc, ExitStack() as ctx:
        big = ctx.enter_context(tc.tile_pool(name="big", bufs=1))
        small = ctx.enter_context(tc.tile_pool(name="small", bufs=1))
        work = ctx.enter_context(tc.tile_pool(name="work", bufs=6))
        flp = ctx.enter_context(tc.tile_pool(name="flp", bufs=NT))
        segp = ctx.enter_context(tc.tile_pool(name="segp", bufs=3))
        psum_m = ctx.enter_context(tc.tile_pool(name="psm", bufs=4, space="PSUM"))
        psum_s = ctx.enter_context(tc.tile_pool(name="pss", bufs=1, space="PSUM"))

        # ---------------- loads ----------------
        qT = big.tile([128, KC, PL], F32R)        # [c128, kc, (q hw)]
        for k in range(KC):
            nc.sync.dma_start(
                qT[:, k, :],
                bass.AP(tensor=q_d.ap().tensor, offset=k * 128 * HW,
                        ap=[[HW, 128], [C * HW, QL], [1, HW]]))
        sn = big.tile([128, KC, M], F32R)         # raw support [c128, kc, (s hw)]
        for k in range(KC):
            nc.sync.dma_start(
                sn[:, k, :],
                bass.AP(tensor=s_d.ap().tensor, offset=k * 128 * HW,
                        ap=[[HW, 128], [C * HW, S], [1, HW]]))
        w1t = small.tile([128, KC, 40], F32R)
        nc.sync.dma_start(w1t, w1t_d.rearrange("(kc p) j -> p kc j", p=128))
        qsel = small.tile([128, NT, QL], F32)
        nc.sync.dma_start(qsel, qsel_d.rearrange("(t p) j -> p t j", p=128))
        b1b = small.tile([128, 40], F32)
        nc.sync.dma_start(b1b, bcast_ap(b1_d, 40))
        w2b = small.tile([128, 40], F32)
        nc.sync.dma_start(w2b, bcast_ap(w2_d, 40))
        b2b = small.tile([128, 1], F32)
        nc.sync.dma_start(b2b, bcast_ap(b2_d, 1))
        ones_bf = small.tile([128, 1], BF16)
        nc.vector.memset(ones_bf, 1.0)
        zero11 = small.tile([128, 1], F32)
        nc.vector.memset(zero11, 0.0)

        # ---------------- support norms ----------------
        # all-ones [128,128] lhsT: column-sum AND broadcast to all partitions
        sq = big.tile([128, KC, M], BF16)
        for k in range(KC):
            nc.scalar.activation(out=sq[:, k, :], in_=sn[:, k, :].bitcast(F32),
                                 func=AF.Square)
        allones = small.tile([128, 128], BF16)
        nc.vector.memset(allones, 1.0)
        rn_bc = big.tile([128, M], F32)
        for j in range(NJ):
            bc_ps = psum_s.tile([128, 512], F32, tag="bc")
            for k in range(KC):
                nc.tensor.matmul(bc_ps[:, :NCH], lhsT=allones,
                                 rhs=sq[:, k, j * NCH:(j + 1) * NCH],
                                 start=(k == 0), stop=(k == KC - 1))
            lnt = work.tile([128, NCH], F32, tag="lnt")
            nc.scalar.activation(out=lnt, in_=bc_ps[:, :NCH], func=AF.Ln,
                                 bias=zero11)
            nc.scalar.activation(out=rn_bc[:, j * NCH:(j + 1) * NCH], in_=lnt,
                                 func=AF.Exp, bias=zero11, scale=-0.5)
        sn_n = big.tile([128, KC, M], F32R)
        for k in range(KC):
            nc.vector.tensor_mul(sn_n[:, k, :], sn[:, k, :].bitcast(F32), rn_bc)

        # ---------------- query norms ----------------
        sqq = big.tile([128, KC, PL], BF16)
        for k in range(KC):
            nc.scalar.activation(out=sqq[:, k, :], in_=qT[:, k, :].bitcast(F32),
                                 func=AF.Square)
        arow = small.tile([128, 896], F32)
        nc.vector.memset(arow, 1.0)
        for j2 in range(2):
            bc_ps = psum_s.tile([128, 512], F32, tag="bc")
            for k in range(KC):
                nc.tensor.matmul(bc_ps[:, :400], lhsT=allones,
                                 rhs=sqq[:, k, j2 * 400:(j2 + 1) * 400],
                                 start=(k == 0), stop=(k == KC - 1))
            nc.scalar.copy(arow[:, j2 * 400:(j2 + 1) * 400], bc_ps[:, :400])
        from concourse.masks import make_identity
        ident = small.tile([128, 128], F32)
        make_identity(nc, ident)
        a2pre = small.tile([128, NT], F32)
        for t in range(NT):
            tr_ps = psum_s.tile([128, 128], F32, tag="tr")
            nc.tensor.transpose(tr_ps, arow[:, t * 128:(t + 1) * 128], ident)
            nc.scalar.copy(a2pre[:, t:t + 1], tr_ps[:, 0:1])
        lnq = small.tile([128, NT], F32)
        nc.scalar.activation(out=lnq, in_=a2pre, func=AF.Ln, bias=zero11)
        a_r = small.tile([128, NT], F32)
        nc.scalar.activation(out=a_r, in_=lnq, func=AF.Exp, bias=zero11,
                             scale=-0.5)

        # ---------------- psi / adaptive threshold ----------------
        pp = small.tile([128, NT], F32)
        for t in range(NT):
            rt = min(128, PL - t * 128)
            hid_ps = psum_s.tile([128, 40], F32, tag="hid")
            for k in range(KC):
                nc.tensor.matmul(hid_ps[:rt, :40],
                                 lhsT=qT[:, k, t * 128:t * 128 + rt],
                                 rhs=w1t[:, k, :],
                                 start=(k == 0), stop=(k == KC - 1))
            hid = work.tile([128, 40], F32, tag="hid_sb")
            nc.vector.scalar_tensor_tensor(
                out=hid[:rt], in0=hid_ps[:rt, :40], scalar=a_r[:rt, t:t + 1],
                in1=b1b[:rt], op0=ALU.mult, op1=ALU.add)
            hl = work.tile([128, 40], F32, tag="hl_sb")
            nc.scalar.activation(out=hl[:rt], in_=hid[:rt], func=AF.Lrelu,
                                 bias=zero11[:rt], scale=1.0, alpha=0.2)
            scr40 = work.tile([128, 40], F32, tag="scr40")
            nc.vector.affine_mul_reduce(
                out=scr40[:rt], accum_out=pp[:rt, t:t + 1],
                in0=hl[:rt], in1=w2b[:rt], scale=1.0, bias=0.0)
        sigp = small.tile([128, NT], F32)
        nc.scalar.activation(out=sigp, in_=pp, func=AF.Sigmoid, bias=b2b,
                             scale=1.0)
        bias_sb = small.tile([128, NT], F32)
        nc.vector.tensor_scalar(out=bias_sb, in0=sigp,
                                scalar1=-ATT_SCALE * VALUE_INTERVAL,
                                scalar2=-ATT_SCALE * FROM_VALUE,
                                op0=ALU.mult, op1=ALU.add)
        scale_sb = small.tile([128, NT], F32)
        nc.vector.tensor_scalar(out=scale_sb, in0=a_r, scalar1=ATT_SCALE,
                                scalar2=None, op0=ALU.mult)

        # ---------------- main loop ----------------
        fl_tiles = []
        for t in range(NT):
            rt = min(128, PL - t * 128)
            l1p = work.tile([128, NJ], F32, tag="l1p")
            nc.vector.memset(l1p, 0.0)
            seg = segp.tile([128, S], F32, tag="seg")
            nc.vector.memset(seg, 0.0)
            for j in range(NJ):
                g_ps = psum_m.tile([128, 512], F32, tag="gps")
                for k in range(KC):
                    nc.tensor.matmul(g_ps[:rt, :NCH],
                                     lhsT=qT[:, k, t * 128:t * 128 + rt],
                                     rhs=sn_n[:, k, j * NCH:(j + 1) * NCH],
                                     start=(k == 0), stop=(k == KC - 1))
                cf = work.tile([128, NCH], F32, tag="cf")
                nc.scalar.activation(out=cf[:rt], in_=g_ps[:rt, :NCH],
                                     func=AF.Sigmoid,
                                     bias=bias_sb[:rt, t:t + 1],
                                     scale=scale_sb[:rt, t:t + 1],
                                     accum_out=l1p[:rt, j:j + 1])
                for s5 in range(NCH // HW):
                    col = j * (NCH // HW) + s5
                    vscr = work.tile([128, HW], F32, tag="vscr")
                    nc.vector.affine_mul_reduce(
                        out=vscr[:rt], accum_out=seg[:rt, col:col + 1],
                        in0=cf[:rt, s5 * HW:(s5 + 1) * HW],
                        in1=g_ps[:rt, s5 * HW:(s5 + 1) * HW],
                        scale=1.0, bias=0.0)
            l1v = work.tile([128, 1], F32, tag="l1v")
            nc.vector.tensor_reduce(out=l1v, in_=l1p,
                                    axis=mybir.AxisListType.X, op=ALU.add)
            nc.vector.tensor_scalar_max(out=l1v, in0=l1v, scalar1=EPS)
            l1r = work.tile([128, 1], F32, tag="l1r")
            nc.vector.reciprocal(l1r, l1v)
            afac = work.tile([128, 1], F32, tag="afac")
            nc.vector.tensor_scalar(out=afac, in0=l1r,
                                    scalar1=a_r[:, t:t + 1], scalar2=None,
                                    op0=ALU.mult)
            flt = flp.tile([128, S], F32, tag="flt")
            nc.vector.tensor_scalar(out=flt, in0=seg, scalar1=afac,
                                    scalar2=None, op0=ALU.mult)
            nc.sync.dma_start(fl_d[t * 128:t * 128 + rt, :], flt[:rt])
            fl_tiles.append(flt)

        # ---------------- final score ----------------
        fs_ps = psum_s.tile([QL, 512], F32, tag="fsps")
        for t in range(NT):
            nc.tensor.matmul(fs_ps[:QL, :S], lhsT=qsel[:, t, :],
                             rhs=fl_tiles[t], start=(t == 0),
                             stop=(t == NT - 1))
        fs_sb = small.tile([QL, S], F32)
        nc.vector.tensor_scalar(out=fs_sb, in0=fs_ps[:QL, :S],
                                scalar1=SCALE / HW, scalar2=None, op0=ALU.mult)
        nc.sync.dma_start(fs_d.ap(), fs_sb)

    nc.compile()
    return nc


def _get_nc():
    if "nc" not in _CACHE:
        _CACHE["nc"] = _build_nc()
    return _CACHE["nc"]


def _qsel_host():
    qsel = np.zeros((NT * 128, QL), dtype=np.float32)
    rows = np.arange(PL)
    qsel[rows, rows // HW] = 1.0
    return qsel


def kernel(query_data, support_data, w1, b1, w2, b2):
    from concourse.bass_utils import run_bass_kernel_spmd

    query_data = np.asarray(query_data, dtype=np.float32)
    support_data = np.asarray(support_data, dtype=np.float32)
    q_resh = np.ascontiguousarray(query_data.reshape(Q, C, HW))
    s_resh = np.ascontiguousarray(support_data.reshape(S, C, HW))
    w1t = np.ascontiguousarray(np.asarray(w1, dtype=np.float32).T)     # [640, 40]
    b1 = np.ascontiguousarray(np.asarray(b1, dtype=np.float32).reshape(40))
    w2v = np.ascontiguousarray(np.asarray(w2, dtype=np.float32).reshape(40))
    b2 = np.ascontiguousarray(np.asarray(b2, dtype=np.float32).reshape(1))
    qsel = _qsel_host()

    nc = _get_nc()
    in_maps = []
    for i in range(NCORES):
        in_maps.append({
            "q": np.ascontiguousarray(q_resh[i * QL:(i + 1) * QL]),
            "s": s_resh, "w1t": w1t, "b1": b1, "w2": w2v, "b2": b2,
            "qsel": qsel,
        })
    res = run_bass_kernel_spmd(nc, in_maps, list(range(NCORES)))
    _CACHE["last_results"] = res

    fl = np.concatenate([r["flocal"] for r in res.results], axis=0)  # [6400, 25]
    fs = np.concatenate([r["fscore"] for r in res.results], axis=0)  # [64, 25]
    final_local = np.ascontiguousarray(
        fl.reshape(Q, HW, S).transpose(0, 2, 1)).astype(np.float32)
    return fs.astype(np.float32), final_local


# revision 6
# speedup vs baseline: 1.0555x; 1.0541x over previous
"""Trainium2 Bass kernel for nn_ATLModule (few-shot cosine-attention scoring).

Strategy: data-parallel over the 64 query images (8 per NeuronCore).
Support tensor + tiny MLP weights replicated on every core.

Per core (q=8 local queries, p=800 query pixels, m=2500 support pixels, c=640):
  - Load query/support in natural [c, pixels] layout (c on partitions).
  - Support/query L2 norms: ACT Square (bf16) + ones-matmul column reduce,
    DRAM round-trip reshape to per-partition layout, sqrt + exact reciprocal.
  - Raw Gram matrix G = qT.T @ sn_normalized via fp32r matmuls (full-rate PE),
    query norm folded into the per-partition ACT scale.
  - cf = Sigmoid(scale_p * G + bias_p) on ScalarE with fused accum -> l1 row sums.
  - Per-support segment sums of cf*G via fused DVE tensor_tensor_reduce.
  - final_local = (1/|q_p|) * seg / max(l1, 1e-12); final_score via 0/1
    selection matmul + *0.3 (mean over 100 pixels * scale 30).
"""
import numpy as np

Q, S, C, H, W = 64, 25, 640, 10, 10
HW = H * W                    # 100
NCORES = 8
QL = Q // NCORES              # 8 queries per core
PL = QL * HW                  # 800 query-pixel rows per core
M = S * HW                    # 2500 support columns
KC = C // 128                 # 5 contraction chunks
NCH = 500                     # support columns per psum chunk (5 supports)
NJ = M // NCH                 # 5 chunks
NT = (PL + 127) // 128        # 7 query-pixel row tiles
SCALE = 30.0
ATT_SCALE = 50.0
FROM_VALUE = 0.5
VALUE_INTERVAL = 0.3
EPS = 1e-12

_CACHE = {}


def _build_nc():
    import concourse.bass as bass
    import concourse.tile as tile
    from concourse import bacc, mybir
    from contextlib import ExitStack

    F32 = mybir.dt.float32
    F32R = mybir.dt.float32r
    BF16 = mybir.dt.bfloat16
    AF = mybir.ActivationFunctionType
    ALU = mybir.AluOpType

    nc = bacc.Bacc("TRN2", target_bir_lowering=False, debug=False,
                   num_devices=NCORES)

    q_d = nc.dram_tensor("q", [QL, C, HW], F32R, kind="ExternalInput")
    s_d = nc.dram_tensor("s", [S, C, HW], F32R, kind="ExternalInput")
    w1t_d = nc.dram_tensor("w1t", [C, 40], F32R, kind="ExternalInput")
    b1_d = nc.dram_tensor("b1", [40], F32, kind="ExternalInput")
    w2_d = nc.dram_tensor("w2", [40], F32, kind="ExternalInput")
    b2_d = nc.dram_tensor("b2", [1], F32, kind="ExternalInput")
    qsel_d = nc.dram_tensor("qsel", [NT * 128, QL], F32, kind="ExternalInput")
    fl_d = nc.dram_tensor("flocal", [PL, S], F32, kind="ExternalOutput")
    fs_d = nc.dram_tensor("fscore", [QL, S], F32, kind="ExternalOutput")

    def bcast_ap(handle, n):
        ap = handle.ap()
        return bass.AP(tensor=ap.tensor, offset=ap.offset, ap=[[0, 128], [1, n]])

    with tile.TileContext(nc) as tc, ExitStack() as ctx:
        big = ctx.enter_context(tc.tile_pool(name="big", bufs=1))
        small = ctx.enter_context(tc.tile_pool(name="small", bufs=1))
        work = ctx.enter_context(tc.tile_pool(name="work", bufs=6))
        flp = ctx.enter_context(tc.tile_pool(name="flp", bufs=NT))
        segp = ctx.enter_context(tc.tile_pool(name="segp", bufs=3))
        psum_m = ctx.enter_context(tc.tile_pool(name="psm", bufs=4, space="PSUM"))
        psum_s = ctx.enter_context(tc.tile_pool(name="pss", bufs=1, space="PSUM"))

        # ---------------- loads ----------------
        qT = big.tile([128, KC, PL], F32R)        # [c128, kc, (q hw)]
        for k in range(KC):
            nc.sync.dma_start(
                qT[:, k, :],
                bass.AP(tensor=q_d.ap().tensor, offset=k * 128 * HW,
                        ap=[[HW, 128], [C * HW, QL], [1, HW]]))
        sn = big.tile([128, KC, M], F32R)         # raw support [c128, kc, (s hw)]
        for k in range(KC):
            nc.sync.dma_start(
                sn[:, k, :],
                bass.AP(tensor=s_d.ap().tensor, offset=k * 128 * HW,
                        ap=[[HW, 128], [C * HW, S], [1, HW]]))
        w1t = small.tile([128, KC, 40], F32R)
        nc.sync.dma_start(w1t, w1t_d.rearrange("(kc p) j -> p kc j", p=128))
        qsel = small.tile([128, NT, QL], F32)
        nc.sync.dma_start(qsel, qsel_d.rearrange("(t p) j -> p t j", p=128))
        b1b = small.tile([128, 40], F32)
        nc.sync.dma_start(b1b, bcast_ap(b1_d, 40))
        w2b = small.tile([128, 40], F32)
        nc.sync.dma_start(w2b, bcast_ap(w2_d, 40))
        b2b = small.tile([128, 1], F32)
        nc.sync.dma_start(b2b, bcast_ap(b2_d, 1))
        ones_bf = small.tile([128, 1], BF16)
        nc.vector.memset(ones_bf, 1.0)
        zero11 = small.tile([128, 1], F32)
        nc.vector.memset(zero11, 0.0)

        # ---------------- support norms ----------------
        # all-ones [128,128] lhsT: column-sum AND broadcast to all partitions
        sq = big.tile([128, KC, M], BF16)
        for k in range(KC):
            nc.scalar.activation(out=sq[:, k, :], in_=sn[:, k, :].bitcast(F32),
                                 func=AF.Square)
        sqq = big.tile([128, KC, PL], BF16)
        for k in range(KC):
            nc.scalar.activation(out=sqq[:, k, :], in_=qT[:, k, :].bitcast(F32),
                                 func=AF.Square)
        allones = small.tile([128, 128], BF16)
        nc.vector.memset(allones, 1.0)
        rn_bc = big.tile([128, M], F32)
        for j in range(NJ):
            bc_ps = psum_s.tile([128, 512], F32, tag="bc")
            for k in range(KC):
                nc.tensor.matmul(bc_ps[:, :NCH], lhsT=allones,
                                 rhs=sq[:, k, j * NCH:(j + 1) * NCH],
                                 start=(k == 0), stop=(k == KC - 1))
            lnt = work.tile([128, NCH], F32, tag="lnt")
            nc.scalar.activation(out=lnt, in_=bc_ps[:, :NCH], func=AF.Ln,
                                 bias=zero11)
            nc.scalar.activation(out=rn_bc[:, j * NCH:(j + 1) * NCH], in_=lnt,
                                 func=AF.Exp, bias=zero11, scale=-0.5)
        sn_n = big.tile([128, KC, M], F32R)
        for k in range(KC):
            nc.vector.tensor_mul(sn_n[:, k, :], sn[:, k, :].bitcast(F32), rn_bc)

        # ---------------- query norms ----------------
        arow = small.tile([128, 896], F32)
        nc.vector.memset(arow, 1.0)
        for j2 in range(2):
            bc_ps = psum_s.tile([128, 512], F32, tag="bc")
            for k in range(KC):
                nc.tensor.matmul(bc_ps[:, :400], lhsT=allones,
                                 rhs=sqq[:, k, j2 * 400:(j2 + 1) * 400],
                                 start=(k == 0), stop=(k == KC - 1))
            nc.scalar.copy(arow[:, j2 * 400:(j2 + 1) * 400], bc_ps[:, :400])
        from concourse.masks import make_identity
        ident = small.tile([128, 128], F32)
        make_identity(nc, ident)
        a2pre = small.tile([128, NT], F32)
        for t in range(NT):
            tr_ps = psum_s.tile([128, 128], F32, tag="tr")
            nc.tensor.transpose(tr_ps, arow[:, t * 128:(t + 1) * 128], ident)
            nc.scalar.copy(a2pre[:, t:t + 1], tr_ps[:, 0:1])
        lnq = small.tile([128, NT], F32)
        nc.scalar.activation(out=lnq, in_=a2pre, func=AF.Ln, bias=zero11)
        a_r = small.tile([128, NT], F32)
        nc.scalar.activation(out=a_r, in_=lnq, func=AF.Exp, bias=zero11,
                             scale=-0.5)

        # ---------------- psi / adaptive threshold ----------------
        pp = small.tile([128, NT], F32)
        for t in range(NT):
            rt = min(128, PL - t * 128)
            hid_ps = psum_s.tile([128, 40], F32, tag="hid")
            for k in range(KC):
                nc.tensor.matmul(hid_ps[:rt, :40],
                                 lhsT=qT[:, k, t * 128:t * 128 + rt],
                                 rhs=w1t[:, k, :],
                                 start=(k == 0), stop=(k == KC - 1))
            hid = work.tile([128, 40], F32, tag="hid_sb")
            nc.vector.scalar_tensor_tensor(
                out=hid[:rt], in0=hid_ps[:rt, :40], scalar=a_r[:rt, t:t + 1],
                in1=b1b[:rt], op0=ALU.mult, op1=ALU.add)
            hl = work.tile([128, 40], F32, tag="hl_sb")
            nc.scalar.activation(out=hl[:rt], in_=hid[:rt], func=AF.Lrelu,
                                 bias=zero11[:rt], scale=1.0, alpha=0.2)
            scr40 = work.tile([128, 40], F32, tag="scr40")
            nc.vector.affine_mul_reduce(
                out=scr40[:rt], accum_out=pp[:rt, t:t + 1],
                in0=hl[:rt], in1=w2b[:rt], scale=1.0, bias=0.0)
        sigp = small.tile([128, NT], F32)
        nc.scalar.activation(out=sigp, in_=pp, func=AF.Sigmoid, bias=b2b,
                             scale=1.0)
        bias_sb = small.tile([128, NT], F32)
        nc.vector.tensor_scalar(out=bias_sb, in0=sigp,
                                scalar1=-ATT_SCALE * VALUE_INTERVAL,
                                scalar2=-ATT_SCALE * FROM_VALUE,
                                op0=ALU.mult, op1=ALU.add)
        scale_sb = small.tile([128, NT], F32)
        nc.vector.tensor_scalar(out=scale_sb, in0=a_r, scalar1=ATT_SCALE,
                                scalar2=None, op0=ALU.mult)

        # ---------------- main loop ----------------
        fl_tiles = []
        for t in range(NT):
            rt = min(128, PL - t * 128)
            l1p = work.tile([128, NJ], F32, tag="l1p")
            nc.vector.memset(l1p, 0.0)
            seg = segp.tile([128, S], F32, tag="seg")
            nc.vector.memset(seg, 0.0)
            for j in range(NJ):
                g_ps = psum_m.tile([128, 512], F32, tag="gps")
                for k in range(KC):
                    nc.tensor.matmul(g_ps[:rt, :NCH],
                                     lhsT=qT[:, k, t * 128:t * 128 + rt],
                                     rhs=sn_n[:, k, j * NCH:(j + 1) * NCH],
                                     start=(k == 0), stop=(k == KC - 1))
                cf = work.tile([128, NCH], F32, tag="cf")
                nc.scalar.activation(out=cf[:rt], in_=g_ps[:rt, :NCH],
                                     func=AF.Sigmoid,
                                     bias=bias_sb[:rt, t:t + 1],
                                     scale=scale_sb[:rt, t:t + 1],
                                     accum_out=l1p[:rt, j:j + 1])
                vmul = work.tile([128, NCH], F32, tag="vmul")
                nc.vector.tensor_mul(vmul[:rt], cf[:rt], g_ps[:rt, :NCH])
                c0 = j * (NCH // HW)
                nc.vector.tensor_reduce(
                    out=seg[:rt, c0:c0 + NCH // HW],
                    in_=vmul[:rt].rearrange("p (s h) -> p s h", h=HW),
                    axis=mybir.AxisListType.X, op=ALU.add)
            l1v = work.tile([128, 1], F32, tag="l1v")
            nc.vector.tensor_reduce(out=l1v, in_=l1p,
                                    axis=mybir.AxisListType.X, op=ALU.add)
            nc.vector.tensor_scalar_max(out=l1v, in0=l1v, scalar1=EPS)
            l1r = work.tile([128, 1], F32, tag="l1r")
            nc.vector.reciprocal(l1r, l1v)
            afac = work.tile([128, 1], F32, tag="afac")
            nc.vector.tensor_scalar(out=afac, in0=l1r,
                                    scalar1=a_r[:, t:t + 1], scalar2=None,
                                    op0=ALU.mult)
            flt = flp.tile([128, S], F32, tag="flt")
            nc.vector.tensor_scalar(out=flt, in0=seg, scalar1=afac,
                                    scalar2=None, op0=ALU.mult)
            nc.sync.dma_start(fl_d[t * 128:t * 128 + rt, :], flt[:rt])
            fl_tiles.append(flt)

        # ---------------- final score ----------------
        fs_ps = psum_s.tile([QL, 512], F32, tag="fsps")
        for t in range(NT):
            nc.tensor.matmul(fs_ps[:QL, :S], lhsT=qsel[:, t, :],
                             rhs=fl_tiles[t], start=(t == 0),
                             stop=(t == NT - 1))
        fs_sb = small.tile([QL, S], F32)
        nc.vector.tensor_scalar(out=fs_sb, in0=fs_ps[:QL, :S],
                                scalar1=SCALE / HW, scalar2=None, op0=ALU.mult)
        nc.sync.dma_start(fs_d.ap(), fs_sb)

    nc.compile()
    return nc


def _get_nc():
    if "nc" not in _CACHE:
        _CACHE["nc"] = _build_nc()
    return _CACHE["nc"]


def _qsel_host():
    qsel = np.zeros((NT * 128, QL), dtype=np.float32)
    rows = np.arange(PL)
    qsel[rows, rows // HW] = 1.0
    return qsel


def kernel(query_data, support_data, w1, b1, w2, b2):
    from concourse.bass_utils import run_bass_kernel_spmd

    query_data = np.asarray(query_data, dtype=np.float32)
    support_data = np.asarray(support_data, dtype=np.float32)
    q_resh = np.ascontiguousarray(query_data.reshape(Q, C, HW))
    s_resh = np.ascontiguousarray(support_data.reshape(S, C, HW))
    w1t = np.ascontiguousarray(np.asarray(w1, dtype=np.float32).T)     # [640, 40]
    b1 = np.ascontiguousarray(np.asarray(b1, dtype=np.float32).reshape(40))
    w2v = np.ascontiguousarray(np.asarray(w2, dtype=np.float32).reshape(40))
    b2 = np.ascontiguousarray(np.asarray(b2, dtype=np.float32).reshape(1))
    qsel = _qsel_host()

    nc = _get_nc()
    in_maps = []
    for i in range(NCORES):
        in_maps.append({
            "q": np.ascontiguousarray(q_resh[i * QL:(i + 1) * QL]),
            "s": s_resh, "w1t": w1t, "b1": b1, "w2": w2v, "b2": b2,
            "qsel": qsel,
        })
    res = run_bass_kernel_spmd(nc, in_maps, list(range(NCORES)))
    _CACHE["last_results"] = res

    fl = np.concatenate([r["flocal"] for r in res.results], axis=0)  # [6400, 25]
    fs = np.concatenate([r["fscore"] for r in res.results], axis=0)  # [64, 25]
    final_local = np.ascontiguousarray(
        fl.reshape(Q, HW, S).transpose(0, 2, 1)).astype(np.float32)
    return fs.astype(np.float32), final_local
